# revision 26
# baseline (speedup 1.0000x reference)
"""AdditiveAttention (Bahdanau) distributed Bass kernel for 8 TRN2 NeuronCores.

Computation (per batch b):
    qc[b,:]   = query[b] @ Wq + bq + bv                       # [512]
    z[b,s,:]  = value[b,s] @ Wv + qc[b]                       # pre-tanh
    score     = tanh(z) @ Wo          (+bo dropped: cancels in softmax)
    align     = softmax(score)        (no max-sub: |score| <= ~23, exp fits f32)
    out[b,:]  = align @ value[b]

Sharding: data-parallel over batch, 4 batches per core, weights replicated.

v3 design:
  - value is relaid out on the HOST to [b, blk, p, hc, s] f32 (h = hc*128+p,
    s_glob = blk*512 + s).  Each per-(b,blk) SWDGE cast-load is 128
    descriptors of 8KB contiguous DRAM reads writing bf16 -- no on-chip
    transpose at all (the xbar-transpose baseline burned ~91us/engine of DMA
    on 256B packets and stalled the PE for ~46us at startup).
  - main mm per (pair, hoc): psum[128ho, 1024s] accumulated per s-half over
    4 k-chunks (Wv stationary, vT moving).  tanh on ACT in [128,1024] spans
    with per-partition bias qcombT (query projection is free).
  - score per pair: psum[1,1024] = sum_hoc Wo[:,hoc].T @ hT[:,hoc,:], lagged
    one pair so the PE never waits on ACT.  Exp reads the score PSUM
    directly (no SBUF score row, no DVE copies), accum_out -> pair totals.
  - context incrementally per s-half, OFF the PE: gpsimd partition-broadcast
    of esc -> [128, 2048], then per hc a DVE scalar_tensor_tensor
    ctx_half[h] = sum_s vT[h,s]*esc[s] (free-dim weighted reduce with
    accumulator output).  Spreading exp/bcast/ctx across the batch leaves
    only the final half's chain (~10us) as the serial tail, vs ~27us for a
    batch-at-the-end chain.
  - output: (ctx_h0+ctx_h1) * (1/total) -> DMA straight to out[b, (hc p)].
"""

import numpy as np

N_CORES = 8
BATCH_TOTAL = 32
B = BATCH_TOTAL // N_CORES  # batches per core
SEQ = 4096
H = 512
HC = H // 128   # 4 hidden chunks
NBLK = 8        # 512-seq blocks
SBLK = SEQ // NBLK
NPR = NBLK // 2  # 1024-seq pairs
PBLK = 2 * SBLK

_cache = {}


def build_nc(b_per_core=B, seq=SEQ):
    import concourse.bass as bass
    import concourse.mybir as mybir
    import concourse.tile as tile
    from concourse import bacc
    from concourse.masks import make_identity

    f32 = mybir.dt.float32
    bf16 = mybir.dt.bfloat16
    fp8 = mybir.dt.float8e4
    AF = mybir.ActivationFunctionType
    ALU = mybir.AluOpType
    AX = mybir.AxisListType
    PM = mybir.MatmulPerfMode

    nblk = seq // SBLK
    npr = nblk // 2

    nc = bacc.Bacc("TRN2", target_bir_lowering=False, debug=False)

    val_d = nc.dram_tensor(
        "value", [b_per_core, nblk, 128, HC, SBLK], f32, kind="ExternalInput"
    ).ap()
    q_d = nc.dram_tensor("query", [b_per_core, H], f32, kind="ExternalInput").ap()
    Wq_d = nc.dram_tensor("Wq", [H, H], f32, kind="ExternalInput").ap()
    bq_d = nc.dram_tensor("bq", [H], f32, kind="ExternalInput").ap()
    Wv_d = nc.dram_tensor("Wv", [H, H], f32, kind="ExternalInput").ap()
    bv_d = nc.dram_tensor("bv", [H], f32, kind="ExternalInput").ap()
    Wo_d = nc.dram_tensor("Wo", [H, 1], f32, kind="ExternalInput").ap()
    bo_d = nc.dram_tensor("bo", [1], f32, kind="ExternalInput").ap()  # unused
    out_d = nc.dram_tensor("out", [b_per_core, H], f32, kind="ExternalOutput").ap()

    # chunked rows (match vT layout h = hc*128 + p): W_sb[p, c, o] = W[c*128+p, o]
    Wv_v = Wv_d.rearrange("(c p) o -> p c o", p=128)
    Wq_v = Wq_d.rearrange("(c p) o -> p c o", p=128)
    Wo_nat_v = Wo_d.rearrange("(r c) one -> r (c one)", c=128)  # [4, 128]
    bq_v = bq_d.rearrange("(r c) -> r c", c=128)                # [4, 128]
    bv_v = bv_d.rearrange("(r c) -> r c", c=128)
    out_v = out_d.rearrange("b (c p) -> b p c", p=128)          # [b, 128, 4]

    with tile.TileContext(nc) as tc:
        with (
            tc.tile_pool(name="weights", bufs=1) as wpool,
            tc.tile_pool(name="vt", bufs=3) as vpool,
        ):
            Wv_sb = wpool.tile([128, HC, H], bf16)
            Wq_sb = wpool.tile([128, HC, H], bf16)
            Wo_sb = wpool.tile([128, HC], bf16)
            qcombT = wpool.tile([128, HC, b_per_core], f32)
            ones_bf = wpool.tile([1, 128], bf16)

            # gpsimd (cast) queue carries ONLY q -> Wq -> Wv -> value blocks,
            # in PE consumption order; everything else (Wo, biases) goes on
            # the scalar HWDGE queue so no setup load queues behind the bulk
            # value stream (that gated the first PE op at ~26us).
            q_nat = wpool.tile([b_per_core, H], bf16)
            nc.gpsimd.dma_start(out=q_nat[:], in_=q_d)
            nc.gpsimd.dma_start(out=Wq_sb[:], in_=Wq_v)
            nc.gpsimd.dma_start(out=Wv_sb[:], in_=Wv_v)

            vts = [None] * b_per_core
            vts[0] = vpool.tile([128, nblk, HC, SBLK], bf16, tag="vt", name="vt")
            for blk in range(4):
                nc.gpsimd.dma_start(out=vts[0][:, blk], in_=val_d[0, blk])

            with (
                tc.tile_pool(name="setup", bufs=1) as spool,
                tc.tile_pool(name="setup_psum", bufs=2, space="PSUM") as spsum,
            ):
                Wo_f32 = spool.tile([4, 128], f32)
                nc.scalar.dma_start(out=Wo_f32[:], in_=Wo_nat_v)
                for blk in range(4, nblk):
                    nc.gpsimd.dma_start(out=vts[0][:, blk], in_=val_d[0, blk])
                nc.gpsimd.memset(ones_bf[:], 1.0)
                Wo_nat = spool.tile([4, 128], bf16)
                nc.vector.tensor_copy(Wo_nat[:], Wo_f32[:])

                id4 = spool.tile([4, 4], bf16)
                make_identity(nc, id4[:])
                id4f = spool.tile([4, 4], f32)
                make_identity(nc, id4f[:])

                # Wo^T: PE-transpose [4,128] -> [128, 4]
                ps_wo = spsum.tile([128, HC], f32, tag="ps_s")
                nc.tensor.matmul(ps_wo[:], Wo_nat[:], id4[:], start=True, stop=True)
                nc.vector.tensor_copy(Wo_sb[:], ps_wo[:])

                # (bq+bv)^T -> [128, 4]
                bq_s = spool.tile([4, 128], f32)
                bv_s = spool.tile([4, 128], f32)
                nc.scalar.dma_start(out=bq_s[:], in_=bq_v)
                nc.scalar.dma_start(out=bv_s[:], in_=bv_v)
                bqv = spool.tile([4, 128], f32)
                nc.vector.tensor_add(bqv[:], bq_s[:], bv_s[:])
                ps_b = spsum.tile([128, HC], f32, tag="ps_s")
                nc.tensor.matmul(ps_b[:], bqv[:], id4f[:], start=True, stop=True)
                bqvT = spool.tile([128, HC], f32)
                nc.vector.tensor_copy(bqvT[:], ps_b[:])

                # q^T chunks: [128, B] per hic
                qT = spool.tile([128, HC, b_per_core], bf16)
                for hic in range(HC):
                    ps_q = spsum.tile([128, b_per_core], f32, tag="ps_s")
                    nc.tensor.matmul(
                        ps_q[:], q_nat[0:b_per_core, 128 * hic:128 * (hic + 1)],
                        id4[0:b_per_core, 0:b_per_core], start=True, stop=True,
                    )
                    nc.vector.tensor_copy(qT[:, hic, :], ps_q[:])

                # qcombT[ho, b] = (q[b] @ Wq)[ho] + bq[ho] + bv[ho]
                for hoc in range(HC):
                    ps_qp = spsum.tile([128, b_per_core], f32, tag="ps_s")
                    for hic in range(HC):
                        nc.tensor.matmul(
                            ps_qp[:], Wq_sb[:, hic, 128 * hoc:128 * (hoc + 1)],
                            qT[:, hic, :], start=(hic == 0), stop=(hic == HC - 1),
                        )
                    nc.scalar.activation(
                        qcombT[:, hoc, :], ps_qp[:], AF.Identity,
                        bias=bqvT[:, hoc:hoc + 1],
                    )

            with (
                tc.tile_pool(name="ht", bufs=3) as hpool,
                tc.tile_pool(name="esc", bufs=2) as epool,
                tc.tile_pool(name="escb", bufs=2) as ebpool,
                tc.tile_pool(name="ttr", bufs=2) as tpool,
                tc.tile_pool(name="small", bufs=12) as smpool,
                tc.tile_pool(name="psum_h", bufs=2, space="PSUM") as psh,
                tc.tile_pool(name="psum_sc", bufs=2, space="PSUM") as pss,
            ):
                HALF = seq // 2
                HBLK = nblk // 2

                # per-batch softmax state, created lazily at first use
                state = {}
                NPARTS = 3  # ctx accumulator slots (halves use 2, b_last 3)

                def get_state(b):
                    if b not in state:
                        esc = epool.tile([1, seq], bf16, tag="esc", name="esc")
                        tots = smpool.tile([1, nblk], f32, tag="tots", name="tots")
                        escb = ebpool.tile([128, seq], bf16, tag="escb", name="escb")
                        ctx_p = smpool.tile([128, HC, NPARTS], f32, tag="ctxp",
                                            name="ctxp")
                        state[b] = (esc, tots, escb, ctx_p)
                    return state[b]

                def emit_score_exp(b, unit, hT):
                    """score matmuls + per-blk Exp from score PSUM."""
                    esc, tots, _, _ = get_state(b)
                    for sh, blk in enumerate(unit):
                        sl = slice(SBLK * blk, SBLK * (blk + 1))
                        ps_sc = pss.tile([1, SBLK], f32, tag="sc")
                        for hoc in range(HC):
                            nc.tensor.matmul(
                                ps_sc[:], Wo_sb[:, hoc:hoc + 1],
                                hT[:, hoc, SBLK * sh:SBLK * (sh + 1)],
                                start=(hoc == 0), stop=(hoc == HC - 1),
                            )
                        nc.scalar.activation(
                            esc[0:1, sl], ps_sc[:], AF.Exp,
                            accum_out=tots[0:1, blk:blk + 1],
                        )

                def emit_bcast(b, blks):
                    esc, _, escb, _ = get_state(b)
                    sl = slice(SBLK * blks[0], SBLK * (blks[-1] + 1))
                    nc.gpsimd.partition_broadcast(
                        escb[:, sl], esc[0:1, sl], channels=128,
                    )

                def emit_ctx(b, part, blks, vt, in1=None):
                    """ctx_p[:,:,part] = sum over blks of vT * esc (per hc)."""
                    _, _, escb, ctx_p = get_state(b)
                    n = len(blks)
                    if in1 is None:
                        in1 = escb[:, SBLK * blks[0]:SBLK * (blks[-1] + 1)]
                    in1 = in1.rearrange("p (k s) -> p k s", s=SBLK)
                    scratch = tpool.tile([128, HBLK, SBLK], bf16, tag="scratch",
                                         name="scratch")
                    for hc in range(HC):
                        nc.vector.scalar_tensor_tensor(
                            out=scratch[:, 0:n, :],
                            in0=vt[:, blks[0]:blks[-1] + 1, hc, :],
                            scalar=1.0,
                            in1=in1,
                            op0=ALU.mult,
                            op1=ALU.mult,
                            accum_out=ctx_p[:, hc, part:part + 1],
                        )

                def emit_final(b, nparts):
                    _, tots, _, ctx_p = get_state(b)
                    ctx = smpool.tile([128, HC], f32, tag="ctx")
                    nc.vector.tensor_reduce(
                        ctx[:], ctx_p[:, :, 0:nparts], AX.X, ALU.add,
                    )
                    tot = smpool.tile([1, 1], f32, tag="tot")
                    nc.vector.tensor_reduce(tot[:], tots[:], AX.X, ALU.add)
                    rec = smpool.tile([1, 1], f32, tag="rec")
                    nc.vector.reciprocal(rec[:], tot[:])
                    rec128 = smpool.tile([128, 1], f32, tag="rec128")
                    nc.gpsimd.partition_broadcast(rec128[:], rec[:], channels=128)
                    outT = smpool.tile([128, HC], f32, tag="outT")
                    nc.vector.tensor_scalar_mul(outT[:], ctx[:], rec128[:])
                    # gpsimd queue is warm (value loads); the sync HWDGE queue
                    # added ~10us of completion latency on the final store
                    nc.gpsimd.dma_start(out=out_v[b], in_=outT[:])
                    del state[b]

                def emit_mms(b, unit, vt, hT, hocs=range(HC)):
                    for hoc in hocs:
                        ph = psh.tile([128, PBLK], f32, tag="ph")
                        for sh, blk in enumerate(unit):
                            for k in range(HC):
                                nc.tensor.matmul(
                                    ph[:, SBLK * sh:SBLK * (sh + 1)],
                                    Wv_sb[:, k, 128 * hoc:128 * (hoc + 1)],
                                    vt[:, blk, k, :],
                                    start=(k == 0), stop=(k == HC - 1),
                                )
                        nc.scalar.activation(
                            hT[:, hoc, :], ph[:], AF.Tanh,
                            bias=qcombT[:, hoc, b:b + 1],
                        )

                UNITS = [(0, 1), (2, 3), (4, 5), (6, 7)]
                last_b = b_per_core - 1

                carry = None  # (b, unit, hT) of previous batch's last unit
                for b in range(b_per_core):
                    if b + 1 < b_per_core:
                        vts[b + 1] = vpool.tile(
                            [128, nblk, HC, SBLK], bf16, tag="vt", name="vt"
                        )

                    prev = None  # (unit, hT) pending score emission (lag 1)
                    for ui, unit in enumerate(UNITS):
                        if b + 1 < b_per_core:
                            for blk in unit:
                                nc.gpsimd.dma_start(
                                    out=vts[b + 1][:, blk], in_=val_d[b + 1, blk]
                                )
                        hT = hpool.tile([128, HC, PBLK], bf16, tag="ht")
                        if b == last_b and ui == 3:
                            # final unit: emit its first hoc, then the early
                            # score for unit 2 so softmax+context of blks 4-5
                            # overlap these matmuls instead of trailing them
                            emit_mms(b, unit, vts[b], hT, hocs=[0])
                            punit, phT = prev
                            emit_score_exp(b, punit, phT)
                            emit_bcast(b, list(punit))
                            emit_ctx(b, 1, [4, 5], vts[b])
                            emit_mms(b, unit, vts[b], hT, hocs=[1, 2, 3])
                            prev = (unit, hT)
                            continue
                        emit_mms(b, unit, vts[b], hT)

                        # one-unit lag so the PE never waits on ACT
                        if prev is not None:
                            punit, phT = prev
                            emit_score_exp(b, punit, phT)
                            emit_bcast(b, list(punit))
                            if punit[-1] == 3:
                                emit_ctx(b, 0, list(range(HBLK)), vts[b])
                        elif carry is not None:
                            cb, cunit, chT = carry
                            emit_score_exp(cb, cunit, chT)
                            emit_bcast(cb, list(cunit))
                            emit_ctx(cb, 1, [4, 5, 6, 7], vts[cb])
                            emit_final(cb, 2)
                            vts[cb] = None
                        prev = (unit, hT)
                    carry = (b, UNITS[-1], hT)

                # tail: last batch's final pair, with PE psum-broadcast of esc
                b, tunit, thT = carry
                esc, _, _, _ = get_state(b)
                emit_score_exp(b, tunit, thT)
                ph_bc = psh.tile([128, PBLK], f32, tag="ph")
                for sh, blk in enumerate(tunit):
                    nc.tensor.matmul(
                        ph_bc[:, SBLK * sh:SBLK * (sh + 1)], ones_bf[:],
                        esc[0:1, SBLK * blk:SBLK * (blk + 1)],
                        start=True, stop=True,
                    )
                emit_ctx(b, 2, list(tunit), vts[b], in1=ph_bc[:])
                emit_final(b, 3)

    nc.compile()
    return nc


def _relayout_value(value_core):
    """[b, seq, H] f32 -> [b, blk, p, hc, s] with h = hc*128+p, sg = blk*512+s."""
    b = value_core.shape[0]
    v = value_core.reshape(b, NBLK, SBLK, HC, 128)
    return np.ascontiguousarray(v.transpose(0, 1, 4, 3, 2))


def make_in_maps(inputs):
    query = np.asarray(inputs["query"], dtype=np.float32)   # [1, 32, 512]
    value = np.asarray(inputs["value"], dtype=np.float32)   # [32, 4096, 512]
    Wq = np.asarray(inputs["Wq"], dtype=np.float32)
    bq = np.asarray(inputs["bq"], dtype=np.float32)
    Wv = np.asarray(inputs["Wv"], dtype=np.float32)
    bv = np.asarray(inputs["bv"], dtype=np.float32)
    Wo = np.asarray(inputs["Wo"], dtype=np.float32)
    bo = np.asarray(inputs["bo"], dtype=np.float32)

    in_maps = []
    for i in range(N_CORES):
        sl = slice(B * i, B * (i + 1))
        in_maps.append({
            "value": _relayout_value(value[sl]),
            "query": np.ascontiguousarray(query[0, sl, :]),
            "Wq": Wq, "bq": bq, "Wv": Wv, "bv": bv, "Wo": Wo, "bo": bo,
        })
    return in_maps


def kernel(**inputs):
    from concourse.bass_utils import run_bass_kernel_spmd

    key = "full"
    if key not in _cache:
        _cache[key] = build_nc()
    nc = _cache[key]

    in_maps = make_in_maps(inputs)
    res = run_bass_kernel_spmd(nc, in_maps, core_ids=list(range(N_CORES)))
    out = np.concatenate([res.results[i]["out"] for i in range(N_CORES)], axis=0)
    return out[:, None, :].astype(np.float32)  # [32, 1, 512]
